# revision 19
# baseline (speedup 1.0000x reference)
"""DetailPooling Trainium2 Bass kernel.

Reference computation (per sample, per channel, image [H=256, W=256]):
  eq2   = depthwise 3x3 binomial blur ([1,2,1] (x) [1,2,1] / 16), replicate pad
  eq56  = ((x - eq2)^2 + 1e-12) ** (2*|lam|)
  eq4   = eq56 + |alpha|
  denom = avgpool2x2-stride1(eq4, edge pad bottom/right) + 1e-8
  out   = avgpool2x2-stride2(x * eq4 / denom)

Sharding: pure data parallel, batch 16 -> 8 cores x 2 samples.
Per-core layout: partitions = (b_local, c) = 2*64 = 128, free dim = H*W.
All stencils run along the free dim (DVE shifted adds); ln/exp/square on ACT.
v2: bf16 intermediates on the DVE ops (2x tensor_tensor rate), pointwise
affine ops offloaded to ACT, fp32 kept on the ln/exp chain.
"""

import os
import numpy as np

N_CORES = 8
B, C, H, W = 16, 64, 256, 256
B_LOC = B // N_CORES          # 2 samples per core
P = B_LOC * C                 # 128 partitions
HT = 16                       # output rows (of H) per tile
N_TILES = H // HT             # 16
HO, WO = H // 2, W // 2

_cache = {}

# cfg "fast": bf16 conv/d path. cfg "safe": fp32 d path (better precision).
CFG = os.environ.get("KERNEL_CFG", "safe")


def _build(cfg=None, rep=1, probe=None):
    import concourse.mybir as mybir
    from concourse import bacc, tile

    cfg = cfg or CFG
    f32 = mybir.dt.float32
    bf16 = mybir.dt.bfloat16
    Alu = mybir.AluOpType
    Act = mybir.ActivationFunctionType

    # conv_dt: blur/d chain; pool_dt: eq56/denominator pool; out_dt: numerator
    conv_dt = bf16 if cfg == "fast" else f32
    pool_dt = bf16 if cfg in ("fast", "mixed", "mixed2") else f32
    out_dt = bf16 if cfg in ("fast", "mixed") else f32

    nc = bacc.Bacc("TRN2", target_bir_lowering=False, debug=False,
                   num_devices=N_CORES)
    x_ap = nc.dram_tensor("x", [P, H * W], f32, kind="ExternalInput").ap()
    lam_ap = nc.dram_tensor("lam", [1, 1], f32, kind="ExternalInput").ap()
    alpha_ap = nc.dram_tensor("alpha", [1, 1], f32, kind="ExternalInput").ap()
    out_ap = nc.dram_tensor("out", [P, HO * WO], f32, kind="ExternalOutput").ap()

    xd = x_ap.rearrange("p (h w) -> p h w", w=W)      # [128, 256, 256]
    od = out_ap.rearrange("p (h w) -> p h w", w=WO)   # [128, 128, 128]

    with tile.TileContext(nc) as tc:
        with tc.tile_pool(name="cpool", bufs=1) as cpool, \
             tc.tile_pool(name="pool", bufs=1) as pool:
            # ---- scalar prep: 2|lam|, 0.25|alpha|, |alpha|+1e-8 ----
            sc_row = cpool.tile([1, 8], f32)
            nc.sync.dma_start(sc_row[0:1, 0:1], lam_ap)
            nc.sync.dma_start(sc_row[0:1, 1:2], alpha_ap)
            nc.scalar.activation(sc_row[0:1, 2:3], sc_row[0:1, 0:1],
                                 Act.Abs, scale=2.0)        # 2|lam|
            nc.scalar.activation(sc_row[0:1, 3:4], sc_row[0:1, 1:2],
                                 Act.Abs)                   # |alpha|
            nc.vector.tensor_scalar_mul(sc_row[0:1, 4:5], sc_row[0:1, 3:4],
                                        0.25)               # 0.25|alpha|
            nc.vector.tensor_scalar_add(sc_row[0:1, 5:6], sc_row[0:1, 3:4],
                                        1e-8)               # |alpha|+1e-8
            scal = cpool.tile([128, 8], f32)
            nc.gpsimd.partition_broadcast(scal[:, :], sc_row[0:1, :])
            la2 = scal[:, 2:3]
            al4 = scal[:, 4:5]
            al8 = scal[:, 5:6]
            eps2 = cpool.tile([128, 1], f32)
            nc.vector.memset(eps2[:], 1e-12)
            lnq = cpool.tile([128, 1], f32)
            nc.vector.memset(lnq[:], float(np.log(0.25)))

            for i_rep in range(rep * N_TILES):
                i = i_rep % N_TILES
                h0 = HT * i
                # x tile rows map to image rows h0-1 .. h0+17 (clamped)
                x_t = pool.tile([P, HT + 3, W], f32, tag="x", bufs=2)
                if i == 0:
                    nc.sync.dma_start(x_t[:, 1:19, :], xd[:, 0:18, :])
                    nc.sync.dma_start(x_t[:, 0:1, :], xd[:, 0:1, :])
                elif i == N_TILES - 1:
                    nc.sync.dma_start(x_t[:, 0:17, :], xd[:, h0 - 1:H, :])
                    nc.sync.dma_start(x_t[:, 17:18, :], xd[:, H - 1:H, :])
                    nc.sync.dma_start(x_t[:, 18:19, :], xd[:, H - 1:H, :])
                else:
                    nc.sync.dma_start(x_t[:, :, :], xd[:, h0 - 1:h0 + 18, :])

                # cast x -> bf16 working copy (ACT)
                if cfg == "fast":
                    xb = pool.tile([P, HT + 3, W], bf16, tag="xb", bufs=2)
                    if probe == "noact":
                        nc.vector.tensor_copy(xb[:], x_t[:])
                    else:
                        nc.scalar.copy(xb[:], x_t[:])
                else:
                    xb = x_t

                w1 = pool.tile([P, HT + 1, W], conv_dt, tag="w1", bufs=2)
                w2 = pool.tile([P, HT + 1, W], conv_dt, tag="w2",
                               bufs=2 if conv_dt == bf16 else 1)
                # vertical [1,2,1]: t rows j=0..16 <-> image rows h0+j
                nc.vector.tensor_tensor(
                    w1[:], xb[:, 0:17, :], xb[:, 2:19, :], Alu.add)
                nc.vector.scalar_tensor_tensor(
                    w1[:], xb[:, 1:18, :], 2.0, w1[:], Alu.mult, Alu.add)
                # horizontal [1,2,1] with replicate pad -> RAW = 16*eq2
                nc.vector.tensor_tensor(
                    w2[:, :, 1:255], w1[:, :, 0:254], w1[:, :, 2:256], Alu.add)
                nc.vector.tensor_tensor(
                    w2[:, :, 0:1], w1[:, :, 0:1], w1[:, :, 1:2], Alu.add)
                nc.vector.tensor_tensor(
                    w2[:, :, 255:256], w1[:, :, 254:255], w1[:, :, 255:256],
                    Alu.add)
                nc.vector.scalar_tensor_tensor(
                    w2[:], w1[:], 2.0, w2[:], Alu.mult, Alu.add)
                # d = x - RAW/16
                nc.vector.scalar_tensor_tensor(
                    w2[:], w2[:], -1.0 / 16.0, xb[:, 1:18, :],
                    Alu.mult, Alu.add)
                # E = eq56 = exp(2|lam| * ln(d^2 + 1e-12)); ln chain in fp32
                eb = pool.tile([P, HT + 1, W], pool_dt, tag="eb",
                               bufs=2 if pool_dt == bf16 else 1)
                if probe == "noact":
                    nc.vector.tensor_copy(eb[:], w2[:])
                else:
                    sf = pool.tile([P, HT + 1, W], f32, tag="sf",
                                   bufs=2 if conv_dt == bf16 else 1)
                    nc.scalar.activation(sf[:], w2[:], Act.Square)
                    nc.scalar.activation(sf[:], sf[:], Act.Ln, bias=eps2[:])
                    nc.scalar.activation(eb[:], sf[:], Act.Exp, scale=la2,
                                         bias=lnq[:])
                # 2x2 stride-1 sum of E (edge pad right/bottom) -> PV
                if conv_dt == pool_dt:
                    p1 = w1
                else:
                    p1 = pool.tile([P, HT + 1, W], pool_dt, tag="p1", bufs=1)
                nc.vector.tensor_tensor(
                    p1[:, :, 0:255], eb[:, :, 0:255], eb[:, :, 1:256], Alu.add)
                nc.vector.tensor_scalar_mul(
                    p1[:, :, 255:256], eb[:, :, 255:256], 2.0)
                pv = pool.tile([P, HT, W], pool_dt, tag="pv")
                if i == N_TILES - 1:
                    nc.vector.tensor_tensor(
                        pv[:, 0:15, :], p1[:, 0:15, :], p1[:, 1:16, :], Alu.add)
                    nc.vector.tensor_scalar_mul(
                        pv[:, 15:16, :], p1[:, 15:16, :], 2.0)
                else:
                    nc.vector.tensor_tensor(
                        pv[:], p1[:, 0:16, :], p1[:, 1:17, :], Alu.add)
                # denom = PV/4 + (|alpha| + 1e-8) on ACT;  R = 1/denom on DVE
                den = pool.tile([P, HT * W], f32, tag="den")
                if probe == "noact" or os.environ.get("KERNEL_DEN", "act") == "dve":
                    nc.vector.tensor_scalar(
                        den[:], pv[:].rearrange("p h w -> p (h w)"),
                        al8, None, Alu.add)
                else:
                    nc.scalar.activation(den[:],
                                         pv[:].rearrange("p h w -> p (h w)"),
                                         Act.Identity, bias=al8)
                if os.environ.get("KERNEL_RECIP", "fast") == "fast":
                    nc.vector.reciprocal_approx_fast(den[:], den[:])
                else:
                    nc.vector.reciprocal(den[:], den[:])
                # M = (0.25*E + 0.25|alpha|) * R   (fused affine+mul)
                mb = pool.tile([P, HT * W], out_dt, tag="mb")
                eb_flat = eb[:].rearrange("p h w -> p (h w)")
                nc.vector.scalar_tensor_tensor(
                    mb[:], eb_flat[:, 0:HT * W], al4, den[:],
                    Alu.add, Alu.mult)
                mb3 = mb[:].rearrange("p (h w) -> p h w", w=W)
                # NUM = M * x
                nc.vector.tensor_tensor(
                    mb3, mb3, xb[:, 1:17, :], Alu.mult)
                # final 2x2 stride-2 sum
                q = pool.tile([P, HT, WO], out_dt, tag="q", bufs=2)
                nc.vector.tensor_tensor(
                    q[:], mb3[:, :, 0:W:2], mb3[:, :, 1:W:2], Alu.add)
                o_t = pool.tile([P, HT // 2, WO], f32, tag="o", bufs=2)
                nc.vector.tensor_tensor(
                    o_t[:], q[:, 0:HT:2, :], q[:, 1:HT:2, :], Alu.add)
                nc.sync.dma_start(od[:, (HT // 2) * i:(HT // 2) * (i + 1), :],
                                  o_t[:])
    nc.compile()
    return nc


def _get_nc():
    if "nc" not in _cache:
        _cache["nc"] = _build()
    return _cache["nc"]


def kernel(x, lam, alpha):
    if not int(os.environ.get("KERNEL_TRACE", "0")):
        os.environ["BASS_NEVER_TRACE"] = "1"
    import concourse.bass_utils as bass_utils

    x = np.ascontiguousarray(np.asarray(x, dtype=np.float32))
    lam = np.asarray(lam, dtype=np.float32).reshape(1, 1)
    alpha = np.asarray(alpha, dtype=np.float32).reshape(1, 1)
    assert x.shape == (B, C, H, W)

    nc = _get_nc()
    in_maps = []
    for i in range(N_CORES):
        shard = x[i * B_LOC:(i + 1) * B_LOC].reshape(P, H * W)
        in_maps.append({"x": np.ascontiguousarray(shard),
                        "lam": lam, "alpha": alpha})

    res = bass_utils.run_bass_kernel_spmd(
        nc, in_maps, core_ids=list(range(N_CORES)),
        trace=bool(int(os.environ.get("KERNEL_TRACE", "0"))))
    _cache["last_results"] = res

    out = np.empty((B, C, HO, WO), dtype=np.float32)
    for i in range(N_CORES):
        out[i * B_LOC:(i + 1) * B_LOC] = \
            res.results[i]["out"].reshape(B_LOC, C, HO, WO)
    return out
